# revision 43
# baseline (speedup 1.0000x reference)
"""Trainium2 Bass kernel for nn_DiscreteDecisionEngine.

Math: the reference computes
    q = tanh(geodesic_weights)            # [1, N, 4], N = 256
    h = L(q) (x)  (quaternion Hamilton product per 4-group)
    logits = h_flat @ W.T + b
The Hamilton product is a block-diagonal (4x4 per group) linear map B(q)
applied to x, so logits = x @ (W @ B)^T + b. We fold W' = W @ B on the
host (tiny: [256,1024] weights) and run a pure GEMM on 8 NeuronCores,
data-parallel over the batch.

The kernel is HBM-traffic-bound, so the host also pre-transposes x into
PE-ready [d-partition, batch-free] tiles and narrows it to fp16 (or
float8e3 with the scale folded into W'), and the device returns fp16
logits-without-bias that the host upcasts + biases. Device work per x
tile [128 rows] is then just 8 accumulating matmuls psum[128,256] +=
xT_k.T @ W'T_k and one DVE cast-copy psum -> fp16. A few zero matmuls
at the start keep the PE busy through its p-state ramp while the first
DMAs land, and the w load is split per contraction chunk so the first
real matmul can begin as soon as chunk 0 arrives.
"""

import os
from contextlib import ExitStack

import ml_dtypes
import numpy as np

import concourse.bass as bass
import concourse.mybir as mybir
import concourse.tile as tile
from concourse import bacc
from concourse.bass import ts
from concourse.bass_utils import run_bass_kernel_spmd

N_CORES = 8
B_FULL = 65536
B_SHARD = B_FULL // N_CORES  # 8192
D = 1024
A = 256  # num actions
KC = D // 128  # 8 contraction chunks
T = B_SHARD // 128  # 64 row tiles per core

_F32 = mybir.dt.float32
_F16 = mybir.dt.float16
_F8 = mybir.dt.float8e3

# tuning knobs (overridable via env for A/B experiments)
_XDT = os.environ.get("K_XDT", "dr")  # f16 | f8 | dr (fp8e4 DoubleRow)
_DR_SX = 8.0  # x scale into e4m3 range
_DR_SW = 64.0  # W scale into e4m3 range
_DR_LOK = 6  # k-chunks with an x_lo slot (of 8)
_X8_SCALE = float(os.environ.get("K_X8_SCALE", "2.0"))
_WARM = int(os.environ.get("K_WARM", "26"))  # PE warm-up matmuls
_MIDWARM = int(os.environ.get("K_MIDWARM", "0"))  # keep-hot matmuls per group
_BHEAD = int(os.environ.get("K_BHEAD", "8"))  # tiles in the early xB load
_BPOS = int(os.environ.get("K_BPOS", "2"))  # load groups before xB-rest
_WFIRST = int(os.environ.get("K_WFIRST", "5"))  # k-chunks in first w load
_WX0 = int(os.environ.get("K_WX0", "2"))  # x tile-0 load before first w load
# early tiles consume a scaled float8e3 copy of w (728 ns load instead of
# 1456) so the fp16 w stream moves off the critical path; their psum copies
# undo the 2^7 scale
_W8TILES = int(os.environ.get("K_W8TILES", "0"))
_W8FIRST = int(os.environ.get("K_W8FIRST", "5"))  # k-chunks in first w8 load
_W16POS = int(os.environ.get("K_W16POS", "4"))  # x load groups before w fp16
_W8SCALE = 128.0
_CSPLIT = int(os.environ.get("K_CSPLIT", "1"))  # column-split all chains
# chain column widths (must sum to A); 128+128 and 85*3+1 both round the
# per-matmul cost down vs a single 256-wide chain
_CCOLS = [int(s) for s in os.environ.get("K_CCOLS", "128,128").split(",")]
_LCOLS = [int(s) for s in os.environ.get("K_LCOLS", "128,128").split(",")]
_LAST_SP = int(os.environ.get("K_LAST_SP", "1"))  # final store on SP ring
# load-group schedule: head groups, mid group size, tail groups
_LHEAD = os.environ.get("K_LHEAD", "2,2")
_LMID = int(os.environ.get("K_LMID", "4"))
_LTAIL = os.environ.get("K_LTAIL", "")
# store-group schedule over the same 64 tiles
_SHEAD = os.environ.get("K_SHEAD", "")
_SMID = int(os.environ.get("K_SMID", "16"))
_STAIL = os.environ.get("K_STAIL", "4,2,1,1")
_BUFS_XIN = int(os.environ.get("K_BUFS_XIN", "8"))
_BUFS_PO = int(os.environ.get("K_BUFS_PO", "6"))
_BUFS_OB = int(os.environ.get("K_BUFS_OB", "3"))
_COPY_ENG = os.environ.get("K_COPY_ENG", "v")  # v | s | alt


def _groups(head, mid, tail):
    head = [int(s) for s in head.split(",") if s]
    tail = [int(s) for s in tail.split(",") if s]
    mid_total = T - sum(head) - sum(tail)
    assert mid_total >= 0, (head, mid, tail)
    rem = mid_total % mid
    sizes = head + ([rem] if rem else []) + [mid] * (mid_total // mid) + tail
    out = []
    t0 = 0
    for g in sizes:
        out.append((t0, g))
        t0 += g
    assert t0 == T
    return out


def _build_nc():
    if _XDT == "dr":
        return _build_nc_dr()
    x_dt = _F8 if _XDT == "f8" else _F16
    nc = bacc.Bacc(None, target_bir_lowering=False)

    # host-pretransposed x: x_dram[p, (t*KC + k)*128 + c] = x[t*128 + c, k*128 + p]
    x = nc.dram_tensor("x", [128, T * KC * 128], x_dt, kind="ExternalInput")
    # w[p, k*A + a] = W'[a, 128*k + p]  (host-prepared, SBUF layout)
    w = nc.dram_tensor("w", [128, KC * A], _F16, kind="ExternalInput")
    w8 = (
        nc.dram_tensor("w8", [128, KC * A], _F8, kind="ExternalInput")
        if _W8TILES > 0
        else None
    )
    # out[c, t*A + a] = logits[t*128 + c, a] - b[a], fp16; host adds bias
    out = nc.dram_tensor("out", [128, T * A], _F16, kind="ExternalOutput")

    with ExitStack() as ctx:
        tc = ctx.enter_context(tile.TileContext(nc))
        const = ctx.enter_context(tc.tile_pool(name="const", bufs=1))
        xin = ctx.enter_context(tc.tile_pool(name="xin", bufs=_BUFS_XIN))
        po = ctx.enter_context(tc.tile_pool(name="po", bufs=_BUFS_PO, space="PSUM"))
        # distinct chain widths get their own small PSUM pools (bufs is
        # per-tag; 8 banks total)
        po_w = {}
        if _CSPLIT:
            widths = sorted(set(_CCOLS + _LCOLS), reverse=True)
            po_w[widths[0]] = po
            for wd in widths[1:]:
                nb = 2 if wd in _CCOLS else 1
                po_w[wd] = ctx.enter_context(
                    tc.tile_pool(name=f"po{wd}", bufs=nb, space="PSUM")
                )
        ob = ctx.enter_context(tc.tile_pool(name="ob", bufs=_BUFS_OB))

        lgroups = _groups(_LHEAD, _LMID, _LTAIL)
        sgroups = _groups(_SHEAD, _SMID, _STAIL)

        # first x tile rides the DMA engines first, then the w chunks, so the
        # PE pipeline starts as early as possible
        tiles = {}

        def load_group(row0, g):
            xg = xin.tile([128, g, KC * 128], x_dt, tag=f"xg{g}")
            src = x[:, bass.ds(row0 * KC * 128, g * KC * 128)]
            nc.sync.dma_start(xg[:], src.rearrange("p (t d) -> p t d", t=g))
            for t in range(g):
                tiles[row0 + t] = (xg, t)

        # w arrives in (up to) two separately-waitable pieces on the same ring
        # as x so the first matmuls only wait for the chunk they consume
        wsplits = []  # (k0, nk, tile)
        if 0 < _WFIRST < KC:
            wsplits.append((0, _WFIRST))
            wsplits.append((_WFIRST, KC - _WFIRST))
        else:
            wsplits.append((0, KC))

        def load_w(k0, nk):
            wt = const.tile([128, nk, A], _F16, tag=f"w{k0}")
            nc.sync.dma_start(
                wt[:],
                w[:, bass.ds(k0 * A, nk * A)].rearrange("p (k a) -> p k a", k=nk),
            )
            return wt

        w_tiles = {}  # k -> (tile, local index)

        def emit_w(k0, nk):
            wt = load_w(k0, nk)
            for k in range(k0, k0 + nk):
                w_tiles[k] = (wt, k - k0)

        w8_tiles = {}
        deferred_w16 = False
        if _W8TILES > 0:
            # w8A, x0, w8B first; the fp16 w rides later in the x stream
            def load_w8(k0, nk):
                wt = const.tile([128, nk, A], _F8, tag=f"w8{k0}")
                nc.sync.dma_start(
                    wt[:],
                    w8[:, bass.ds(k0 * A, nk * A)].rearrange(
                        "p (k a) -> p k a", k=nk
                    ),
                )
                for k in range(k0, k0 + nk):
                    w8_tiles[k] = (wt, k - k0)

            load_w8(0, _W8FIRST)
            load_group(*lgroups[0])
            if _W8FIRST < KC:
                load_w8(_W8FIRST, KC - _W8FIRST)
            deferred_w16 = True
        elif _WX0 == 2 and len(wsplits) == 2:
            # wA, x tile 0, wB: the PE start is gated by x0 while the later
            # k-chunks stream in just ahead of their first use
            emit_w(*wsplits[0])
            load_group(*lgroups[0])
            emit_w(*wsplits[1])
        else:
            if _WX0:
                load_group(*lgroups[0])
            for k0, nk in wsplits:
                emit_w(k0, nk)
            if not _WX0:
                load_group(*lgroups[0])

        # PE p-state warm-up: zero matmuls (DVE memsets the operand) that
        # execute while the first loads are in flight, so real matmuls hit
        # the full-speed clock immediately
        if _WARM > 0:
            wn = _CCOLS[0] if _CSPLIT else A
            zwarm = const.tile([128, max(wn, 128)], _F16)
            nc.vector.memset(zwarm[:], 0.0)
            if _CSPLIT:
                pw = po_w[wn].tile([128, wn], _F32, tag=f"po{wn}")
            else:
                pw = po.tile([128, wn], _F32, tag="p_out")
            for _ in range(_WARM):
                nc.tensor.matmul(
                    pw[:], lhsT=zwarm[:, ts(0, 128)], rhs=zwarm[:, :wn],
                    start=True, stop=True,
                )

        for gi, (row0, g) in enumerate(lgroups[1:], start=1):
            if deferred_w16 and gi == _W16POS:
                emit_w(0, KC)
            load_group(row0, g)
        if deferred_w16 and len(lgroups) <= _W16POS:
            emit_w(0, KC)

        def copy_out(dst_ap, src_ap, trow, salt=0):
            if trow < _W8TILES:
                # undo the w8 2^7 host scale while casting psum -> fp16
                nc.vector.tensor_scalar_mul(dst_ap, src_ap, 1.0 / _W8SCALE)
            elif _COPY_ENG == "s" or (_COPY_ENG == "alt" and (trow + salt) % 2):
                nc.scalar.copy(out=dst_ap, in_=src_ap)
            else:
                nc.vector.tensor_copy(out=dst_ap, in_=src_ap)

        for si, (srow0, sg) in enumerate(sgroups):
            og = ob.tile([128, sg, A], _F16, tag=f"ob{sg}")
            for j in range(sg):
                trow = srow0 + j
                xg, t = tiles[trow]
                if _CSPLIT:
                    # column-split chains: narrower chains round the
                    # per-matmul cost down, and each chain's copy overlaps
                    # the next chain's matmuls
                    c0 = 0
                    wsel = w8_tiles if trow < _W8TILES else w_tiles
                    cols = _LCOLS if trow == T - 1 else _CCOLS
                    for h, cw in enumerate(cols):
                        p_half = po_w[cw].tile([128, cw], _F32, tag=f"po{cw}")
                        for k in range(KC):
                            wt, kl = wsel[k]
                            nc.tensor.matmul(
                                p_half[:],
                                lhsT=xg[:, t, ts(k, 128)],
                                rhs=wt[:, kl, bass.ds(c0, cw)],
                                start=(k == 0),
                                stop=(k == KC - 1),
                            )
                        copy_out(og[:, j, bass.ds(c0, cw)], p_half[:], trow, h)
                        c0 += cw
                else:
                    p_out = po.tile([128, A], _F32, tag="p_out")
                    wsel = w8_tiles if trow < _W8TILES else w_tiles
                    for k in range(KC):
                        wt, kl = wsel[k]
                        nc.tensor.matmul(
                            p_out[:],
                            lhsT=xg[:, t, ts(k, 128)],
                            rhs=wt[:, kl, :],
                            start=(k == 0),
                            stop=(k == KC - 1),
                        )
                    copy_out(og[:, j, :], p_out[:], trow, 0)
            dst = out[:, bass.ds(srow0 * A, sg * A)]
            ring = nc.sync if (_LAST_SP and si == len(sgroups) - 1) else nc.scalar
            ring.dma_start(dst.rearrange("p (t a) -> p t a", t=sg), og[:])

    nc.finalize()
    return nc


def _build_nc_dr():
    """fp8e4 DoubleRow kernel: each DoubleRow matmul contracts 2x128 slots at
    0.5 cycles/row. Slots carry (x_hi, x_lo) of the SAME 128 dims against a
    duplicated W_hi (full x precision in one instruction), plus depth-packed
    x_hi chains against W_lo to reconstruct W. x_lo ships for the first
    _DR_LOK k-chunks only (1.75 B/elem); scales undone on the host."""
    E4 = mybir.dt.float8e4
    DR = mybir.MatmulPerfMode.DoubleRow
    LOK = _DR_LOK
    HIK = KC - LOK
    nc = bacc.Bacc(None, target_bir_lowering=False)

    xA = nc.dram_tensor("xA", [128, T * LOK * 2 * 128], E4, kind="ExternalInput")
    xB = nc.dram_tensor("xB", [128, T * HIK * 128], E4, kind="ExternalInput")
    wA = nc.dram_tensor("wA", [128, LOK * 2 * A], E4, kind="ExternalInput")
    wB = nc.dram_tensor("wB", [128, HIK * A], E4, kind="ExternalInput")
    wC = nc.dram_tensor("wC", [128, (KC // 2) * 2 * A], E4, kind="ExternalInput")
    out = nc.dram_tensor("out", [128, T * A], _F16, kind="ExternalOutput")

    with ExitStack() as ctx:
        tc = ctx.enter_context(tile.TileContext(nc))
        const = ctx.enter_context(tc.tile_pool(name="const", bufs=1))
        xin = ctx.enter_context(tc.tile_pool(name="xin", bufs=_BUFS_XIN))
        po = ctx.enter_context(tc.tile_pool(name="po", bufs=_BUFS_PO, space="PSUM"))
        pod = (
            ctx.enter_context(tc.tile_pool(name="pod", bufs=1, space="PSUM"))
            if _MIDWARM > 0
            else None
        )
        ob = ctx.enter_context(tc.tile_pool(name="ob", bufs=_BUFS_OB))

        lgroups = _groups(_LHEAD, _LMID, _LTAIL)
        sgroups = _groups(_SHEAD, _SMID, _STAIL)

        tiles = {}
        btiles = {}

        def load_group(row0, g):
            tA = xin.tile([128, g, LOK, 2, 128], E4, tag=f"xa{g}")
            nc.sync.dma_start(
                tA[:],
                xA[:, bass.ds(row0 * LOK * 256, g * LOK * 256)].rearrange(
                    "p (t k s c) -> p t k s c", t=g, k=LOK, s=2, c=128
                ),
            )
            for t in range(g):
                tiles[row0 + t] = (tA, t)

        def load_b(row0, n):
            if n <= 0:
                return
            # the tiny hi-only k6..7 regions ride in bulk DMAs (held in SBUF
            # for the whole run) instead of wasting a HWDGE slot per group
            tB = const.tile([128, n, HIK, 128], E4, tag=f"xball{row0}")
            nc.sync.dma_start(
                tB[:],
                xB[:, bass.ds(row0 * HIK * 128, n * HIK * 128)].rearrange(
                    "p (t k c) -> p t k c", t=n, k=HIK, c=128
                ),
            )
            for t in range(n):
                btiles[row0 + t] = (tB, t)

        load_group(*lgroups[0])
        load_b(0, _BHEAD)
        wA_sb = const.tile([128, LOK, 2, A], E4)
        nc.sync.dma_start(wA_sb[:], wA.rearrange("p (k s a) -> p k s a", k=LOK, s=2))
        wB_sb = const.tile([128, HIK, A], E4)
        nc.sync.dma_start(wB_sb[:], wB.rearrange("p (k a) -> p k a", k=HIK))
        wC_sb = const.tile([128, KC // 2, 2, A], E4)
        nc.sync.dma_start(wC_sb[:], wC.rearrange("p (j s a) -> p j s a", j=KC // 2, s=2))

        if _WARM > 0:
            zwarm = const.tile([128, A], _F16)
            nc.vector.memset(zwarm[:], 0.0)
            pw = po.tile([128, A], _F32, tag="p_out")
            for _ in range(_WARM):
                nc.tensor.matmul(
                    pw[:], lhsT=zwarm[:, ts(0, 128)], rhs=zwarm[:],
                    start=True, stop=True,
                )

        for gi, (row0, g) in enumerate(lgroups[1:], start=1):
            if gi == _BPOS:
                load_b(_BHEAD, T - _BHEAD)
            load_group(row0, g)
        if len(lgroups) <= _BPOS:
            load_b(_BHEAD, T - _BHEAD)

        for si, (srow0, sg) in enumerate(sgroups):
            og = ob.tile([128, sg, A], _F16, tag=f"ob{sg}")
            for j in range(sg):
                trow = srow0 + j
                tA, t = tiles[trow]
                tB, tb = btiles[trow]
                p_out = po.tile([128, A], _F32, tag="p_out")
                # (x_hi + x_lo) @ W_hi for k0..LOK-1: slots are hi/lo of the
                # same dims, rhs slots duplicate W_hi
                for k in range(LOK):
                    nc.tensor.matmul(
                        p_out[:], lhsT=tA[:, t, k, :, :], rhs=wA_sb[:, k, :, :],
                        start=(k == 0), stop=False, perf_mode=DR,
                    )
                # x_hi @ W_hi for the remaining k, depth-packed 2 chunks/inst
                nc.tensor.matmul(
                    p_out[:], lhsT=tB[:, tb, :, :], rhs=wB_sb[:],
                    start=False, stop=False, perf_mode=DR,
                )
                # x_hi @ W_lo over all k, depth-packed pairs
                for pr in range(KC // 2):
                    if 2 * pr + 1 < LOK:
                        lhsT = tA[:, t, ts(pr, 2), 0, :]
                    else:
                        lhsT = tB[:, tb, :, :]
                    nc.tensor.matmul(
                        p_out[:], lhsT=lhsT, rhs=wC_sb[:, pr, :, :],
                        start=False, stop=(pr == KC // 2 - 1), perf_mode=DR,
                    )
                nc.vector.tensor_copy(out=og[:, j, :], in_=p_out[:])
            dst = out[:, bass.ds(srow0 * A, sg * A)]
            ring = nc.sync if (_LAST_SP and si == len(sgroups) - 1) else nc.scalar
            ring.dma_start(dst.rearrange("p (t a) -> p t a", t=sg), og[:])
            if _MIDWARM > 0 and si < len(sgroups) - 1:
                # dependency-free matmuls run whenever the PE would idle
                # waiting for the next loads, keeping the p-state ramp hot
                pd = pod.tile([128, A], _F32, tag="pd")
                for _ in range(_MIDWARM):
                    nc.tensor.matmul(
                        pd[:], lhsT=zwarm[:, ts(0, 128)], rhs=zwarm[:],
                        start=True, stop=True,
                    )

    nc.finalize()
    return nc


_NC_CACHE = None


def _get_nc():
    global _NC_CACHE
    if _NC_CACHE is None:
        _NC_CACHE = _build_nc()
    return _NC_CACHE


def _fold_weights(geodesic_weights: np.ndarray, W: np.ndarray) -> np.ndarray:
    """W' = W @ blockdiag(L(tanh(g))^T per 4-group), in float64."""
    q = np.tanh(geodesic_weights.astype(np.float64))[0]  # [N, 4]
    w_, i_, j_, k_ = q[:, 0], q[:, 1], q[:, 2], q[:, 3]
    n = q.shape[0]
    M = np.empty((n, 4, 4), dtype=np.float64)  # y_r = sum_s M[n, r, s] x_s
    M[:, 0] = np.stack([w_, -i_, -j_, -k_], axis=-1)
    M[:, 1] = np.stack([i_, w_, -k_, j_], axis=-1)
    M[:, 2] = np.stack([j_, k_, w_, -i_], axis=-1)
    M[:, 3] = np.stack([k_, -j_, i_, w_], axis=-1)
    W4 = W.astype(np.float64).reshape(A, n, 4)  # [a, n, r]
    Wp = np.einsum("anr,nrs->ans", W4, M).reshape(A, D)
    return Wp  # [a, d] float64


def _pack_dr(x, Wp):
    """Host packing for the DoubleRow kernel (see _build_nc_dr)."""
    E4 = ml_dtypes.float8_e4m3
    LOK = _DR_LOK
    HIK = KC - LOK
    xs = x * np.float32(_DR_SX)
    xh = xs.astype(E4).astype(np.float32)
    xl = (xs - xh).astype(E4)
    xh8 = xh.astype(E4)
    # [core, t, c, k, p]
    xh_r = xh8.reshape(N_CORES, T, 128, KC, 128)
    xl_r = xl.reshape(N_CORES, T, 128, KC, 128)
    # A: [core, p, t, k(<LOK), s, c]
    xA = np.ascontiguousarray(
        np.stack([xh_r[:, :, :, :LOK, :], xl_r[:, :, :, :LOK, :]], axis=4)
        .transpose(0, 5, 1, 3, 4, 2)
        .reshape(N_CORES, 128, T * LOK * 2 * 128)
    )
    xB = np.ascontiguousarray(
        xh_r[:, :, :, LOK:, :]
        .transpose(0, 4, 1, 3, 2)
        .reshape(N_CORES, 128, T * HIK * 128)
    )
    ws = (Wp * _DR_SW).astype(np.float32)
    wh = ws.astype(E4).astype(np.float32)
    wl = (ws - wh).astype(E4)
    wh8 = wh.astype(E4)
    whk = wh8.T.reshape(KC, 128, A)  # [k, p, a]
    wlk = wl.T.reshape(KC, 128, A)
    wA = np.ascontiguousarray(
        np.broadcast_to(whk[:LOK, None], (LOK, 2, 128, A))
        .transpose(2, 0, 1, 3)
        .reshape(128, LOK * 2 * A)
    )
    wB = np.ascontiguousarray(
        whk[LOK:].transpose(1, 0, 2).reshape(128, HIK * A)
    )
    wC = np.ascontiguousarray(
        wlk.reshape(KC // 2, 2, 128, A).transpose(2, 0, 1, 3).reshape(128, -1)
    )
    return xA, xB, wA, wB, wC


def kernel(x, geodesic_weights, W, b, **_unused):
    x = np.asarray(x, dtype=np.float32)
    Wp = _fold_weights(np.asarray(geodesic_weights), np.asarray(W))
    if _XDT == "dr":
        xA, xB, wA, wB, wC = _pack_dr(x, Wp.astype(np.float32))
        nc = _get_nc()
        in_maps = [
            {"xA": xA[c], "xB": xB[c], "wA": wA, "wB": wB, "wC": wC}
            for c in range(N_CORES)
        ]
        res = run_bass_kernel_spmd(
            nc,
            in_maps,
            core_ids=list(range(N_CORES)),
            trace=bool(int(os.environ.get("KERNEL_TRACE", "0"))),
        )
        bf = np.asarray(b, dtype=np.float32)
        inv = np.float32(1.0 / (_DR_SX * _DR_SW))
        out = np.empty((B_FULL, A), dtype=np.float32)
        for c, r in enumerate(res.results):
            o = r["out"].reshape(128, T, A).transpose(1, 0, 2).reshape(B_SHARD, A)
            out[c * B_SHARD : (c + 1) * B_SHARD] = o.astype(np.float32) * inv + bf
        return out
    if _XDT == "f8":
        # global scale keeps x in e3m4's normal range; undone inside W'
        x_dev_full = (x * _X8_SCALE).astype(ml_dtypes.float8_e3m4)
        Wp = Wp / _X8_SCALE
    else:
        x_dev_full = x.astype(np.float16)
    # device layouts (see _build_nc)
    w_dev = np.ascontiguousarray(
        Wp.T.reshape(KC, 128, A).transpose(1, 0, 2).reshape(128, KC * A)
    ).astype(np.float16)
    # x[core] -> [p, t, k, c]
    xt = np.ascontiguousarray(
        x_dev_full.reshape(N_CORES, T, 128, KC, 128)
        .transpose(0, 4, 1, 3, 2)
        .reshape(N_CORES, 128, T * KC * 128)
    )

    nc = _get_nc()
    in_maps = [{"x": xt[c], "w": w_dev} for c in range(N_CORES)]
    if _W8TILES > 0:
        w8_dev = (w_dev.astype(np.float32) * _W8SCALE).astype(
            ml_dtypes.float8_e3m4
        )
        for m in in_maps:
            m["w8"] = w8_dev
    res = run_bass_kernel_spmd(
        nc,
        in_maps,
        core_ids=list(range(N_CORES)),
        trace=bool(int(os.environ.get("KERNEL_TRACE", "0"))),
    )
    bf = np.asarray(b, dtype=np.float32)
    out = np.empty((B_FULL, A), dtype=np.float32)
    for c, r in enumerate(res.results):
        o = r["out"].reshape(128, T, A).transpose(1, 0, 2).reshape(B_SHARD, A)
        out[c * B_SHARD : (c + 1) * B_SHARD] = o.astype(np.float32) + bf
    return out


# revision 45
# speedup vs baseline: 1.0086x; 1.0086x over previous
"""Trainium2 Bass kernel for nn_DiscreteDecisionEngine.

Math: the reference computes
    q = tanh(geodesic_weights)            # [1, N, 4], N = 256
    h = L(q) (x)  (quaternion Hamilton product per 4-group)
    logits = h_flat @ W.T + b
The Hamilton product is a block-diagonal (4x4 per group) linear map B(q)
applied to x, so logits = x @ (W @ B)^T + b. We fold W' = W @ B on the
host (tiny: [256,1024] weights) and run a pure GEMM on 8 NeuronCores,
data-parallel over the batch.

The kernel is HBM-traffic-bound, so the host also pre-transposes x into
PE-ready [d-partition, batch-free] tiles and narrows it to fp16 (or
float8e3 with the scale folded into W'), and the device returns fp16
logits-without-bias that the host upcasts + biases. Device work per x
tile [128 rows] is then just 8 accumulating matmuls psum[128,256] +=
xT_k.T @ W'T_k and one DVE cast-copy psum -> fp16. A few zero matmuls
at the start keep the PE busy through its p-state ramp while the first
DMAs land, and the w load is split per contraction chunk so the first
real matmul can begin as soon as chunk 0 arrives.
"""

import os
from contextlib import ExitStack

import ml_dtypes
import numpy as np

import concourse.bass as bass
import concourse.mybir as mybir
import concourse.tile as tile
from concourse import bacc
from concourse.bass import ts
from concourse.bass_utils import run_bass_kernel_spmd

N_CORES = 8
B_FULL = 65536
B_SHARD = B_FULL // N_CORES  # 8192
D = 1024
A = 256  # num actions
KC = D // 128  # 8 contraction chunks
T = B_SHARD // 128  # 64 row tiles per core

_F32 = mybir.dt.float32
_F16 = mybir.dt.float16
_F8 = mybir.dt.float8e3

# tuning knobs (overridable via env for A/B experiments)
_XDT = os.environ.get("K_XDT", "dr")  # f16 | f8 | dr (fp8e4 DoubleRow)
_DR_SX = 8.0  # x scale into e4m3 range
_DR_SW = 64.0  # W scale into e4m3 range
_DR_LOK = int(os.environ.get("K_LOK", "6"))  # k-chunks with an x_lo slot
_X8_SCALE = float(os.environ.get("K_X8_SCALE", "2.0"))
_WARM = int(os.environ.get("K_WARM", "26"))  # PE warm-up matmuls
_MIDWARM = int(os.environ.get("K_MIDWARM", "0"))  # keep-hot matmuls per group
_BHEAD = int(os.environ.get("K_BHEAD", "8"))  # tiles in the early xB load
_BPOS = int(os.environ.get("K_BPOS", "2"))  # load groups before xB-rest
_WFIRST = int(os.environ.get("K_WFIRST", "5"))  # k-chunks in first w load
_WX0 = int(os.environ.get("K_WX0", "2"))  # x tile-0 load before first w load
# early tiles consume a scaled float8e3 copy of w (728 ns load instead of
# 1456) so the fp16 w stream moves off the critical path; their psum copies
# undo the 2^7 scale
_W8TILES = int(os.environ.get("K_W8TILES", "0"))
_W8FIRST = int(os.environ.get("K_W8FIRST", "5"))  # k-chunks in first w8 load
_W16POS = int(os.environ.get("K_W16POS", "4"))  # x load groups before w fp16
_W8SCALE = 128.0
_CSPLIT = int(os.environ.get("K_CSPLIT", "1"))  # column-split all chains
# chain column widths (must sum to A); 128+128 and 85*3+1 both round the
# per-matmul cost down vs a single 256-wide chain
_CCOLS = [int(s) for s in os.environ.get("K_CCOLS", "128,128").split(",")]
_LCOLS = [int(s) for s in os.environ.get("K_LCOLS", "128,128").split(",")]
_LAST_SP = int(os.environ.get("K_LAST_SP", "1"))  # final store on SP ring
# load-group schedule: head groups, mid group size, tail groups
_LHEAD = os.environ.get("K_LHEAD", "2,2")
_LMID = int(os.environ.get("K_LMID", "4"))
_LTAIL = os.environ.get("K_LTAIL", "")
# store-group schedule over the same 64 tiles
_SHEAD = os.environ.get("K_SHEAD", "")
_SMID = int(os.environ.get("K_SMID", "16"))
_STAIL = os.environ.get("K_STAIL", "4,2,1,1")
_BUFS_XIN = int(os.environ.get("K_BUFS_XIN", "8"))
_BUFS_PO = int(os.environ.get("K_BUFS_PO", "6"))
_BUFS_OB = int(os.environ.get("K_BUFS_OB", "3"))
_COPY_ENG = os.environ.get("K_COPY_ENG", "v")  # v | s | alt


def _groups(head, mid, tail):
    head = [int(s) for s in head.split(",") if s]
    tail = [int(s) for s in tail.split(",") if s]
    mid_total = T - sum(head) - sum(tail)
    assert mid_total >= 0, (head, mid, tail)
    rem = mid_total % mid
    sizes = head + ([rem] if rem else []) + [mid] * (mid_total // mid) + tail
    out = []
    t0 = 0
    for g in sizes:
        out.append((t0, g))
        t0 += g
    assert t0 == T
    return out


def _build_nc():
    if _XDT == "dr":
        return _build_nc_dr()
    x_dt = _F8 if _XDT == "f8" else _F16
    nc = bacc.Bacc(None, target_bir_lowering=False)

    # host-pretransposed x: x_dram[p, (t*KC + k)*128 + c] = x[t*128 + c, k*128 + p]
    x = nc.dram_tensor("x", [128, T * KC * 128], x_dt, kind="ExternalInput")
    # w[p, k*A + a] = W'[a, 128*k + p]  (host-prepared, SBUF layout)
    w = nc.dram_tensor("w", [128, KC * A], _F16, kind="ExternalInput")
    w8 = (
        nc.dram_tensor("w8", [128, KC * A], _F8, kind="ExternalInput")
        if _W8TILES > 0
        else None
    )
    # out[c, t*A + a] = logits[t*128 + c, a] - b[a], fp16; host adds bias
    out = nc.dram_tensor("out", [128, T * A], _F16, kind="ExternalOutput")

    with ExitStack() as ctx:
        tc = ctx.enter_context(tile.TileContext(nc))
        const = ctx.enter_context(tc.tile_pool(name="const", bufs=1))
        xin = ctx.enter_context(tc.tile_pool(name="xin", bufs=_BUFS_XIN))
        po = ctx.enter_context(tc.tile_pool(name="po", bufs=_BUFS_PO, space="PSUM"))
        # distinct chain widths get their own small PSUM pools (bufs is
        # per-tag; 8 banks total)
        po_w = {}
        if _CSPLIT:
            widths = sorted(set(_CCOLS + _LCOLS), reverse=True)
            po_w[widths[0]] = po
            for wd in widths[1:]:
                nb = 2 if wd in _CCOLS else 1
                po_w[wd] = ctx.enter_context(
                    tc.tile_pool(name=f"po{wd}", bufs=nb, space="PSUM")
                )
        ob = ctx.enter_context(tc.tile_pool(name="ob", bufs=_BUFS_OB))

        lgroups = _groups(_LHEAD, _LMID, _LTAIL)
        sgroups = _groups(_SHEAD, _SMID, _STAIL)

        # first x tile rides the DMA engines first, then the w chunks, so the
        # PE pipeline starts as early as possible
        tiles = {}

        def load_group(row0, g):
            xg = xin.tile([128, g, KC * 128], x_dt, tag=f"xg{g}")
            src = x[:, bass.ds(row0 * KC * 128, g * KC * 128)]
            nc.sync.dma_start(xg[:], src.rearrange("p (t d) -> p t d", t=g))
            for t in range(g):
                tiles[row0 + t] = (xg, t)

        # w arrives in (up to) two separately-waitable pieces on the same ring
        # as x so the first matmuls only wait for the chunk they consume
        wsplits = []  # (k0, nk, tile)
        if 0 < _WFIRST < KC:
            wsplits.append((0, _WFIRST))
            wsplits.append((_WFIRST, KC - _WFIRST))
        else:
            wsplits.append((0, KC))

        def load_w(k0, nk):
            wt = const.tile([128, nk, A], _F16, tag=f"w{k0}")
            nc.sync.dma_start(
                wt[:],
                w[:, bass.ds(k0 * A, nk * A)].rearrange("p (k a) -> p k a", k=nk),
            )
            return wt

        w_tiles = {}  # k -> (tile, local index)

        def emit_w(k0, nk):
            wt = load_w(k0, nk)
            for k in range(k0, k0 + nk):
                w_tiles[k] = (wt, k - k0)

        w8_tiles = {}
        deferred_w16 = False
        if _W8TILES > 0:
            # w8A, x0, w8B first; the fp16 w rides later in the x stream
            def load_w8(k0, nk):
                wt = const.tile([128, nk, A], _F8, tag=f"w8{k0}")
                nc.sync.dma_start(
                    wt[:],
                    w8[:, bass.ds(k0 * A, nk * A)].rearrange(
                        "p (k a) -> p k a", k=nk
                    ),
                )
                for k in range(k0, k0 + nk):
                    w8_tiles[k] = (wt, k - k0)

            load_w8(0, _W8FIRST)
            load_group(*lgroups[0])
            if _W8FIRST < KC:
                load_w8(_W8FIRST, KC - _W8FIRST)
            deferred_w16 = True
        elif _WX0 == 2 and len(wsplits) == 2:
            # wA, x tile 0, wB: the PE start is gated by x0 while the later
            # k-chunks stream in just ahead of their first use
            emit_w(*wsplits[0])
            load_group(*lgroups[0])
            emit_w(*wsplits[1])
        else:
            if _WX0:
                load_group(*lgroups[0])
            for k0, nk in wsplits:
                emit_w(k0, nk)
            if not _WX0:
                load_group(*lgroups[0])

        # PE p-state warm-up: zero matmuls (DVE memsets the operand) that
        # execute while the first loads are in flight, so real matmuls hit
        # the full-speed clock immediately
        if _WARM > 0:
            wn = _CCOLS[0] if _CSPLIT else A
            zwarm = const.tile([128, max(wn, 128)], _F16)
            nc.vector.memset(zwarm[:], 0.0)
            if _CSPLIT:
                pw = po_w[wn].tile([128, wn], _F32, tag=f"po{wn}")
            else:
                pw = po.tile([128, wn], _F32, tag="p_out")
            for _ in range(_WARM):
                nc.tensor.matmul(
                    pw[:], lhsT=zwarm[:, ts(0, 128)], rhs=zwarm[:, :wn],
                    start=True, stop=True,
                )

        for gi, (row0, g) in enumerate(lgroups[1:], start=1):
            if deferred_w16 and gi == _W16POS:
                emit_w(0, KC)
            load_group(row0, g)
        if deferred_w16 and len(lgroups) <= _W16POS:
            emit_w(0, KC)

        def copy_out(dst_ap, src_ap, trow, salt=0):
            if trow < _W8TILES:
                # undo the w8 2^7 host scale while casting psum -> fp16
                nc.vector.tensor_scalar_mul(dst_ap, src_ap, 1.0 / _W8SCALE)
            elif _COPY_ENG == "s" or (_COPY_ENG == "alt" and (trow + salt) % 2):
                nc.scalar.copy(out=dst_ap, in_=src_ap)
            else:
                nc.vector.tensor_copy(out=dst_ap, in_=src_ap)

        for si, (srow0, sg) in enumerate(sgroups):
            og = ob.tile([128, sg, A], _F16, tag=f"ob{sg}")
            for j in range(sg):
                trow = srow0 + j
                xg, t = tiles[trow]
                if _CSPLIT:
                    # column-split chains: narrower chains round the
                    # per-matmul cost down, and each chain's copy overlaps
                    # the next chain's matmuls
                    c0 = 0
                    wsel = w8_tiles if trow < _W8TILES else w_tiles
                    cols = _LCOLS if trow == T - 1 else _CCOLS
                    for h, cw in enumerate(cols):
                        p_half = po_w[cw].tile([128, cw], _F32, tag=f"po{cw}")
                        for k in range(KC):
                            wt, kl = wsel[k]
                            nc.tensor.matmul(
                                p_half[:],
                                lhsT=xg[:, t, ts(k, 128)],
                                rhs=wt[:, kl, bass.ds(c0, cw)],
                                start=(k == 0),
                                stop=(k == KC - 1),
                            )
                        copy_out(og[:, j, bass.ds(c0, cw)], p_half[:], trow, h)
                        c0 += cw
                else:
                    p_out = po.tile([128, A], _F32, tag="p_out")
                    wsel = w8_tiles if trow < _W8TILES else w_tiles
                    for k in range(KC):
                        wt, kl = wsel[k]
                        nc.tensor.matmul(
                            p_out[:],
                            lhsT=xg[:, t, ts(k, 128)],
                            rhs=wt[:, kl, :],
                            start=(k == 0),
                            stop=(k == KC - 1),
                        )
                    copy_out(og[:, j, :], p_out[:], trow, 0)
            dst = out[:, bass.ds(srow0 * A, sg * A)]
            ring = nc.sync if (_LAST_SP and si == len(sgroups) - 1) else nc.scalar
            ring.dma_start(dst.rearrange("p (t a) -> p t a", t=sg), og[:])

    nc.finalize()
    return nc


def _build_nc_dr():
    """fp8e4 DoubleRow kernel: each DoubleRow matmul contracts 2x128 slots at
    0.5 cycles/row. Slots carry (x_hi, x_lo) of the SAME 128 dims against a
    duplicated W_hi (full x precision in one instruction), plus depth-packed
    x_hi chains against W_lo to reconstruct W. x_lo ships for the first
    _DR_LOK k-chunks only (1.75 B/elem); scales undone on the host."""
    E4 = mybir.dt.float8e4
    DR = mybir.MatmulPerfMode.DoubleRow
    LOK = _DR_LOK
    HIK = KC - LOK
    nc = bacc.Bacc(None, target_bir_lowering=False)

    xA = nc.dram_tensor("xA", [128, T * LOK * 2 * 128], E4, kind="ExternalInput")
    xB = nc.dram_tensor("xB", [128, T * HIK * 128], E4, kind="ExternalInput")
    wA = nc.dram_tensor("wA", [128, LOK * A], E4, kind="ExternalInput")
    wB = nc.dram_tensor("wB", [128, HIK * A], E4, kind="ExternalInput")
    wC = nc.dram_tensor("wC", [128, (KC // 2) * 2 * A], E4, kind="ExternalInput")
    out = nc.dram_tensor("out", [128, T * A], _F16, kind="ExternalOutput")

    with ExitStack() as ctx:
        tc = ctx.enter_context(tile.TileContext(nc))
        const = ctx.enter_context(tc.tile_pool(name="const", bufs=1))
        xin = ctx.enter_context(tc.tile_pool(name="xin", bufs=_BUFS_XIN))
        po = ctx.enter_context(tc.tile_pool(name="po", bufs=_BUFS_PO, space="PSUM"))
        pod = (
            ctx.enter_context(tc.tile_pool(name="pod", bufs=1, space="PSUM"))
            if _MIDWARM > 0
            else None
        )
        ob = ctx.enter_context(tc.tile_pool(name="ob", bufs=_BUFS_OB))

        lgroups = _groups(_LHEAD, _LMID, _LTAIL)
        sgroups = _groups(_SHEAD, _SMID, _STAIL)

        tiles = {}
        btiles = {}

        def load_group(row0, g):
            tA = xin.tile([128, g, LOK, 2, 128], E4, tag=f"xa{g}")
            nc.sync.dma_start(
                tA[:],
                xA[:, bass.ds(row0 * LOK * 256, g * LOK * 256)].rearrange(
                    "p (t k s c) -> p t k s c", t=g, k=LOK, s=2, c=128
                ),
            )
            for t in range(g):
                tiles[row0 + t] = (tA, t)

        def load_b(row0, n):
            if n <= 0:
                return
            # the tiny hi-only k6..7 regions ride in bulk DMAs (held in SBUF
            # for the whole run) instead of wasting a HWDGE slot per group
            tB = const.tile([128, n, HIK, 128], E4, tag=f"xball{row0}")
            nc.sync.dma_start(
                tB[:],
                xB[:, bass.ds(row0 * HIK * 128, n * HIK * 128)].rearrange(
                    "p (t k c) -> p t k c", t=n, k=HIK, c=128
                ),
            )
            for t in range(n):
                btiles[row0 + t] = (tB, t)

        load_group(*lgroups[0])
        load_b(0, _BHEAD)
        wA_sb = const.tile([128, LOK, 1, A], E4)
        nc.sync.dma_start(wA_sb[:], wA.rearrange("p (k s a) -> p k s a", k=LOK, s=1))
        wB_sb = const.tile([128, HIK, A], E4)
        nc.sync.dma_start(wB_sb[:], wB.rearrange("p (k a) -> p k a", k=HIK))
        wC_sb = const.tile([128, KC // 2, 2, A], E4)
        nc.sync.dma_start(wC_sb[:], wC.rearrange("p (j s a) -> p j s a", j=KC // 2, s=2))

        if _WARM > 0:
            zwarm = const.tile([128, A], _F16)
            nc.vector.memset(zwarm[:], 0.0)
            pw = po.tile([128, A], _F32, tag="p_out")
            for _ in range(_WARM):
                nc.tensor.matmul(
                    pw[:], lhsT=zwarm[:, ts(0, 128)], rhs=zwarm[:],
                    start=True, stop=True,
                )

        for gi, (row0, g) in enumerate(lgroups[1:], start=1):
            if gi == _BPOS:
                load_b(_BHEAD, T - _BHEAD)
            load_group(row0, g)
        if len(lgroups) <= _BPOS:
            load_b(_BHEAD, T - _BHEAD)

        for si, (srow0, sg) in enumerate(sgroups):
            og = ob.tile([128, sg, A], _F16, tag=f"ob{sg}")
            for j in range(sg):
                trow = srow0 + j
                tA, t = tiles[trow]
                tB, tb = btiles[trow]
                p_out = po.tile([128, A], _F32, tag="p_out")
                # (x_hi + x_lo) @ W_hi for k0..LOK-1: slots are hi/lo of the
                # same dims, rhs slots duplicate W_hi
                for k in range(LOK):
                    nc.tensor.matmul(
                        p_out[:], lhsT=tA[:, t, k, :, :],
                        rhs=wA_sb[:, k, :, :].broadcast_to([128, 2, A]),
                        start=(k == 0), stop=False, perf_mode=DR,
                    )
                # x_hi @ W_hi for the remaining k, depth-packed 2 chunks/inst
                nc.tensor.matmul(
                    p_out[:], lhsT=tB[:, tb, :, :], rhs=wB_sb[:],
                    start=False, stop=False, perf_mode=DR,
                )
                # x_hi @ W_lo over all k, depth-packed pairs
                for pr in range(KC // 2):
                    if 2 * pr + 1 < LOK:
                        lhsT = tA[:, t, ts(pr, 2), 0, :]
                    else:
                        lhsT = tB[:, tb, :, :]
                    nc.tensor.matmul(
                        p_out[:], lhsT=lhsT, rhs=wC_sb[:, pr, :, :],
                        start=False, stop=(pr == KC // 2 - 1), perf_mode=DR,
                    )
                nc.vector.tensor_copy(out=og[:, j, :], in_=p_out[:])
            dst = out[:, bass.ds(srow0 * A, sg * A)]
            ring = nc.sync if (_LAST_SP and si == len(sgroups) - 1) else nc.scalar
            ring.dma_start(dst.rearrange("p (t a) -> p t a", t=sg), og[:])
            if _MIDWARM > 0 and si < len(sgroups) - 1:
                # dependency-free matmuls run whenever the PE would idle
                # waiting for the next loads, keeping the p-state ramp hot
                pd = pod.tile([128, A], _F32, tag="pd")
                for _ in range(_MIDWARM):
                    nc.tensor.matmul(
                        pd[:], lhsT=zwarm[:, ts(0, 128)], rhs=zwarm[:],
                        start=True, stop=True,
                    )

    nc.finalize()
    return nc


_NC_CACHE = None


def _get_nc():
    global _NC_CACHE
    if _NC_CACHE is None:
        _NC_CACHE = _build_nc()
    return _NC_CACHE


def _fold_weights(geodesic_weights: np.ndarray, W: np.ndarray) -> np.ndarray:
    """W' = W @ blockdiag(L(tanh(g))^T per 4-group), in float64."""
    q = np.tanh(geodesic_weights.astype(np.float64))[0]  # [N, 4]
    w_, i_, j_, k_ = q[:, 0], q[:, 1], q[:, 2], q[:, 3]
    n = q.shape[0]
    M = np.empty((n, 4, 4), dtype=np.float64)  # y_r = sum_s M[n, r, s] x_s
    M[:, 0] = np.stack([w_, -i_, -j_, -k_], axis=-1)
    M[:, 1] = np.stack([i_, w_, -k_, j_], axis=-1)
    M[:, 2] = np.stack([j_, k_, w_, -i_], axis=-1)
    M[:, 3] = np.stack([k_, -j_, i_, w_], axis=-1)
    W4 = W.astype(np.float64).reshape(A, n, 4)  # [a, n, r]
    Wp = np.einsum("anr,nrs->ans", W4, M).reshape(A, D)
    return Wp  # [a, d] float64


def _pack_dr(x, Wp):
    """Host packing for the DoubleRow kernel (see _build_nc_dr)."""
    E4 = ml_dtypes.float8_e4m3
    LOK = _DR_LOK
    HIK = KC - LOK
    xs = x * np.float32(_DR_SX)
    xh = xs.astype(E4).astype(np.float32)
    xl = (xs - xh).astype(E4)
    xh8 = xh.astype(E4)
    # [core, t, c, k, p]
    xh_r = xh8.reshape(N_CORES, T, 128, KC, 128)
    xl_r = xl.reshape(N_CORES, T, 128, KC, 128)
    # A: [core, p, t, k(<LOK), s, c]
    xA = np.ascontiguousarray(
        np.stack([xh_r[:, :, :, :LOK, :], xl_r[:, :, :, :LOK, :]], axis=4)
        .transpose(0, 5, 1, 3, 4, 2)
        .reshape(N_CORES, 128, T * LOK * 2 * 128)
    )
    xB = np.ascontiguousarray(
        xh_r[:, :, :, LOK:, :]
        .transpose(0, 4, 1, 3, 2)
        .reshape(N_CORES, 128, T * HIK * 128)
    )
    ws = (Wp * _DR_SW).astype(np.float32)
    wh = ws.astype(E4).astype(np.float32)
    wl = (ws - wh).astype(E4)
    wh8 = wh.astype(E4)
    whk = wh8.T.reshape(KC, 128, A)  # [k, p, a]
    wlk = wl.T.reshape(KC, 128, A)
    wA = np.ascontiguousarray(
        whk[:LOK].transpose(1, 0, 2).reshape(128, LOK * A)
    )
    wB = np.ascontiguousarray(
        whk[LOK:].transpose(1, 0, 2).reshape(128, HIK * A)
    )
    wC = np.ascontiguousarray(
        wlk.reshape(KC // 2, 2, 128, A).transpose(2, 0, 1, 3).reshape(128, -1)
    )
    return xA, xB, wA, wB, wC


def kernel(x, geodesic_weights, W, b, **_unused):
    x = np.asarray(x, dtype=np.float32)
    Wp = _fold_weights(np.asarray(geodesic_weights), np.asarray(W))
    if _XDT == "dr":
        xA, xB, wA, wB, wC = _pack_dr(x, Wp.astype(np.float32))
        nc = _get_nc()
        in_maps = [
            {"xA": xA[c], "xB": xB[c], "wA": wA, "wB": wB, "wC": wC}
            for c in range(N_CORES)
        ]
        res = run_bass_kernel_spmd(
            nc,
            in_maps,
            core_ids=list(range(N_CORES)),
            trace=bool(int(os.environ.get("KERNEL_TRACE", "0"))),
        )
        bf = np.asarray(b, dtype=np.float32)
        inv = np.float32(1.0 / (_DR_SX * _DR_SW))
        out = np.empty((B_FULL, A), dtype=np.float32)
        for c, r in enumerate(res.results):
            o = r["out"].reshape(128, T, A).transpose(1, 0, 2).reshape(B_SHARD, A)
            out[c * B_SHARD : (c + 1) * B_SHARD] = o.astype(np.float32) * inv + bf
        return out
    if _XDT == "f8":
        # global scale keeps x in e3m4's normal range; undone inside W'
        x_dev_full = (x * _X8_SCALE).astype(ml_dtypes.float8_e3m4)
        Wp = Wp / _X8_SCALE
    else:
        x_dev_full = x.astype(np.float16)
    # device layouts (see _build_nc)
    w_dev = np.ascontiguousarray(
        Wp.T.reshape(KC, 128, A).transpose(1, 0, 2).reshape(128, KC * A)
    ).astype(np.float16)
    # x[core] -> [p, t, k, c]
    xt = np.ascontiguousarray(
        x_dev_full.reshape(N_CORES, T, 128, KC, 128)
        .transpose(0, 4, 1, 3, 2)
        .reshape(N_CORES, 128, T * KC * 128)
    )

    nc = _get_nc()
    in_maps = [{"x": xt[c], "w": w_dev} for c in range(N_CORES)]
    if _W8TILES > 0:
        w8_dev = (w_dev.astype(np.float32) * _W8SCALE).astype(
            ml_dtypes.float8_e3m4
        )
        for m in in_maps:
            m["w8"] = w8_dev
    res = run_bass_kernel_spmd(
        nc,
        in_maps,
        core_ids=list(range(N_CORES)),
        trace=bool(int(os.environ.get("KERNEL_TRACE", "0"))),
    )
    bf = np.asarray(b, dtype=np.float32)
    out = np.empty((B_FULL, A), dtype=np.float32)
    for c, r in enumerate(res.results):
        o = r["out"].reshape(128, T, A).transpose(1, 0, 2).reshape(B_SHARD, A)
        out[c * B_SHARD : (c + 1) * B_SHARD] = o.astype(np.float32) + bf
    return out


# revision 46
# speedup vs baseline: 1.0104x; 1.0018x over previous
"""Trainium2 Bass kernel for nn_DiscreteDecisionEngine.

Math: the reference computes
    q = tanh(geodesic_weights)            # [1, N, 4], N = 256
    h = L(q) (x)  (quaternion Hamilton product per 4-group)
    logits = h_flat @ W.T + b
The Hamilton product is a block-diagonal (4x4 per group) linear map B(q)
applied to x, so logits = x @ (W @ B)^T + b. We fold W' = W @ B on the
host (tiny: [256,1024] weights) and run a pure GEMM on 8 NeuronCores,
data-parallel over the batch.

The kernel is HBM-traffic-bound, so the host also pre-transposes x into
PE-ready [d-partition, batch-free] tiles and narrows it to fp16 (or
float8e3 with the scale folded into W'), and the device returns fp16
logits-without-bias that the host upcasts + biases. Device work per x
tile [128 rows] is then just 8 accumulating matmuls psum[128,256] +=
xT_k.T @ W'T_k and one DVE cast-copy psum -> fp16. A few zero matmuls
at the start keep the PE busy through its p-state ramp while the first
DMAs land, and the w load is split per contraction chunk so the first
real matmul can begin as soon as chunk 0 arrives.
"""

import os
from contextlib import ExitStack

import ml_dtypes
import numpy as np

import concourse.bass as bass
import concourse.mybir as mybir
import concourse.tile as tile
from concourse import bacc
from concourse.bass import ts
from concourse.bass_utils import run_bass_kernel_spmd

N_CORES = 8
B_FULL = 65536
B_SHARD = B_FULL // N_CORES  # 8192
D = 1024
A = 256  # num actions
KC = D // 128  # 8 contraction chunks
T = B_SHARD // 128  # 64 row tiles per core

_F32 = mybir.dt.float32
_F16 = mybir.dt.float16
_F8 = mybir.dt.float8e3

# tuning knobs (overridable via env for A/B experiments)
_XDT = os.environ.get("K_XDT", "dr")  # f16 | f8 | dr (fp8e4 DoubleRow)
_DR_SX = 8.0  # x scale into e4m3 range
_DR_SW = 64.0  # W scale into e4m3 range
_DR_LOK = int(os.environ.get("K_LOK", "6"))  # k-chunks with an x_lo slot
_X8_SCALE = float(os.environ.get("K_X8_SCALE", "2.0"))
_WARM = int(os.environ.get("K_WARM", "26"))  # PE warm-up matmuls
_MIDWARM = int(os.environ.get("K_MIDWARM", "0"))  # keep-hot matmuls per group
_BHEAD = int(os.environ.get("K_BHEAD", "12"))  # tiles in the early xB load
_BPOS = int(os.environ.get("K_BPOS", "2"))  # load groups before xB-rest
_WFIRST = int(os.environ.get("K_WFIRST", "5"))  # k-chunks in first w load
_WX0 = int(os.environ.get("K_WX0", "2"))  # x tile-0 load before first w load
# early tiles consume a scaled float8e3 copy of w (728 ns load instead of
# 1456) so the fp16 w stream moves off the critical path; their psum copies
# undo the 2^7 scale
_W8TILES = int(os.environ.get("K_W8TILES", "0"))
_W8FIRST = int(os.environ.get("K_W8FIRST", "5"))  # k-chunks in first w8 load
_W16POS = int(os.environ.get("K_W16POS", "4"))  # x load groups before w fp16
_W8SCALE = 128.0
_CSPLIT = int(os.environ.get("K_CSPLIT", "1"))  # column-split all chains
# chain column widths (must sum to A); 128+128 and 85*3+1 both round the
# per-matmul cost down vs a single 256-wide chain
_CCOLS = [int(s) for s in os.environ.get("K_CCOLS", "128,128").split(",")]
_LCOLS = [int(s) for s in os.environ.get("K_LCOLS", "128,128").split(",")]
_LAST_SP = int(os.environ.get("K_LAST_SP", "1"))  # final store on SP ring
# load-group schedule: head groups, mid group size, tail groups
_LHEAD = os.environ.get("K_LHEAD", "2,2")
_LMID = int(os.environ.get("K_LMID", "4"))
_LTAIL = os.environ.get("K_LTAIL", "")
# store-group schedule over the same 64 tiles
_SHEAD = os.environ.get("K_SHEAD", "")
_SMID = int(os.environ.get("K_SMID", "16"))
_STAIL = os.environ.get("K_STAIL", "4,2,1,1,1")
_BUFS_XIN = int(os.environ.get("K_BUFS_XIN", "8"))
_BUFS_PO = int(os.environ.get("K_BUFS_PO", "6"))
_BUFS_OB = int(os.environ.get("K_BUFS_OB", "3"))
_COPY_ENG = os.environ.get("K_COPY_ENG", "v")  # v | s | alt


def _groups(head, mid, tail):
    head = [int(s) for s in head.split(",") if s]
    tail = [int(s) for s in tail.split(",") if s]
    mid_total = T - sum(head) - sum(tail)
    assert mid_total >= 0, (head, mid, tail)
    rem = mid_total % mid
    sizes = head + ([rem] if rem else []) + [mid] * (mid_total // mid) + tail
    out = []
    t0 = 0
    for g in sizes:
        out.append((t0, g))
        t0 += g
    assert t0 == T
    return out


def _build_nc():
    if _XDT == "dr":
        return _build_nc_dr()
    x_dt = _F8 if _XDT == "f8" else _F16
    nc = bacc.Bacc(None, target_bir_lowering=False)

    # host-pretransposed x: x_dram[p, (t*KC + k)*128 + c] = x[t*128 + c, k*128 + p]
    x = nc.dram_tensor("x", [128, T * KC * 128], x_dt, kind="ExternalInput")
    # w[p, k*A + a] = W'[a, 128*k + p]  (host-prepared, SBUF layout)
    w = nc.dram_tensor("w", [128, KC * A], _F16, kind="ExternalInput")
    w8 = (
        nc.dram_tensor("w8", [128, KC * A], _F8, kind="ExternalInput")
        if _W8TILES > 0
        else None
    )
    # out[c, t*A + a] = logits[t*128 + c, a] - b[a], fp16; host adds bias
    out = nc.dram_tensor("out", [128, T * A], _F16, kind="ExternalOutput")

    with ExitStack() as ctx:
        tc = ctx.enter_context(tile.TileContext(nc))
        const = ctx.enter_context(tc.tile_pool(name="const", bufs=1))
        xin = ctx.enter_context(tc.tile_pool(name="xin", bufs=_BUFS_XIN))
        po = ctx.enter_context(tc.tile_pool(name="po", bufs=_BUFS_PO, space="PSUM"))
        # distinct chain widths get their own small PSUM pools (bufs is
        # per-tag; 8 banks total)
        po_w = {}
        if _CSPLIT:
            widths = sorted(set(_CCOLS + _LCOLS), reverse=True)
            po_w[widths[0]] = po
            for wd in widths[1:]:
                nb = 2 if wd in _CCOLS else 1
                po_w[wd] = ctx.enter_context(
                    tc.tile_pool(name=f"po{wd}", bufs=nb, space="PSUM")
                )
        ob = ctx.enter_context(tc.tile_pool(name="ob", bufs=_BUFS_OB))

        lgroups = _groups(_LHEAD, _LMID, _LTAIL)
        sgroups = _groups(_SHEAD, _SMID, _STAIL)

        # first x tile rides the DMA engines first, then the w chunks, so the
        # PE pipeline starts as early as possible
        tiles = {}

        def load_group(row0, g):
            xg = xin.tile([128, g, KC * 128], x_dt, tag=f"xg{g}")
            src = x[:, bass.ds(row0 * KC * 128, g * KC * 128)]
            nc.sync.dma_start(xg[:], src.rearrange("p (t d) -> p t d", t=g))
            for t in range(g):
                tiles[row0 + t] = (xg, t)

        # w arrives in (up to) two separately-waitable pieces on the same ring
        # as x so the first matmuls only wait for the chunk they consume
        wsplits = []  # (k0, nk, tile)
        if 0 < _WFIRST < KC:
            wsplits.append((0, _WFIRST))
            wsplits.append((_WFIRST, KC - _WFIRST))
        else:
            wsplits.append((0, KC))

        def load_w(k0, nk):
            wt = const.tile([128, nk, A], _F16, tag=f"w{k0}")
            nc.sync.dma_start(
                wt[:],
                w[:, bass.ds(k0 * A, nk * A)].rearrange("p (k a) -> p k a", k=nk),
            )
            return wt

        w_tiles = {}  # k -> (tile, local index)

        def emit_w(k0, nk):
            wt = load_w(k0, nk)
            for k in range(k0, k0 + nk):
                w_tiles[k] = (wt, k - k0)

        w8_tiles = {}
        deferred_w16 = False
        if _W8TILES > 0:
            # w8A, x0, w8B first; the fp16 w rides later in the x stream
            def load_w8(k0, nk):
                wt = const.tile([128, nk, A], _F8, tag=f"w8{k0}")
                nc.sync.dma_start(
                    wt[:],
                    w8[:, bass.ds(k0 * A, nk * A)].rearrange(
                        "p (k a) -> p k a", k=nk
                    ),
                )
                for k in range(k0, k0 + nk):
                    w8_tiles[k] = (wt, k - k0)

            load_w8(0, _W8FIRST)
            load_group(*lgroups[0])
            if _W8FIRST < KC:
                load_w8(_W8FIRST, KC - _W8FIRST)
            deferred_w16 = True
        elif _WX0 == 2 and len(wsplits) == 2:
            # wA, x tile 0, wB: the PE start is gated by x0 while the later
            # k-chunks stream in just ahead of their first use
            emit_w(*wsplits[0])
            load_group(*lgroups[0])
            emit_w(*wsplits[1])
        else:
            if _WX0:
                load_group(*lgroups[0])
            for k0, nk in wsplits:
                emit_w(k0, nk)
            if not _WX0:
                load_group(*lgroups[0])

        # PE p-state warm-up: zero matmuls (DVE memsets the operand) that
        # execute while the first loads are in flight, so real matmuls hit
        # the full-speed clock immediately
        if _WARM > 0:
            wn = _CCOLS[0] if _CSPLIT else A
            zwarm = const.tile([128, max(wn, 128)], _F16)
            nc.vector.memset(zwarm[:], 0.0)
            if _CSPLIT:
                pw = po_w[wn].tile([128, wn], _F32, tag=f"po{wn}")
            else:
                pw = po.tile([128, wn], _F32, tag="p_out")
            for _ in range(_WARM):
                nc.tensor.matmul(
                    pw[:], lhsT=zwarm[:, ts(0, 128)], rhs=zwarm[:, :wn],
                    start=True, stop=True,
                )

        for gi, (row0, g) in enumerate(lgroups[1:], start=1):
            if deferred_w16 and gi == _W16POS:
                emit_w(0, KC)
            load_group(row0, g)
        if deferred_w16 and len(lgroups) <= _W16POS:
            emit_w(0, KC)

        def copy_out(dst_ap, src_ap, trow, salt=0):
            if trow < _W8TILES:
                # undo the w8 2^7 host scale while casting psum -> fp16
                nc.vector.tensor_scalar_mul(dst_ap, src_ap, 1.0 / _W8SCALE)
            elif _COPY_ENG == "s" or (_COPY_ENG == "alt" and (trow + salt) % 2):
                nc.scalar.copy(out=dst_ap, in_=src_ap)
            else:
                nc.vector.tensor_copy(out=dst_ap, in_=src_ap)

        for si, (srow0, sg) in enumerate(sgroups):
            og = ob.tile([128, sg, A], _F16, tag=f"ob{sg}")
            for j in range(sg):
                trow = srow0 + j
                xg, t = tiles[trow]
                if _CSPLIT:
                    # column-split chains: narrower chains round the
                    # per-matmul cost down, and each chain's copy overlaps
                    # the next chain's matmuls
                    c0 = 0
                    wsel = w8_tiles if trow < _W8TILES else w_tiles
                    cols = _LCOLS if trow == T - 1 else _CCOLS
                    for h, cw in enumerate(cols):
                        p_half = po_w[cw].tile([128, cw], _F32, tag=f"po{cw}")
                        for k in range(KC):
                            wt, kl = wsel[k]
                            nc.tensor.matmul(
                                p_half[:],
                                lhsT=xg[:, t, ts(k, 128)],
                                rhs=wt[:, kl, bass.ds(c0, cw)],
                                start=(k == 0),
                                stop=(k == KC - 1),
                            )
                        copy_out(og[:, j, bass.ds(c0, cw)], p_half[:], trow, h)
                        c0 += cw
                else:
                    p_out = po.tile([128, A], _F32, tag="p_out")
                    wsel = w8_tiles if trow < _W8TILES else w_tiles
                    for k in range(KC):
                        wt, kl = wsel[k]
                        nc.tensor.matmul(
                            p_out[:],
                            lhsT=xg[:, t, ts(k, 128)],
                            rhs=wt[:, kl, :],
                            start=(k == 0),
                            stop=(k == KC - 1),
                        )
                    copy_out(og[:, j, :], p_out[:], trow, 0)
            dst = out[:, bass.ds(srow0 * A, sg * A)]
            ring = nc.sync if (_LAST_SP and si == len(sgroups) - 1) else nc.scalar
            ring.dma_start(dst.rearrange("p (t a) -> p t a", t=sg), og[:])

    nc.finalize()
    return nc


def _build_nc_dr():
    """fp8e4 DoubleRow kernel: each DoubleRow matmul contracts 2x128 slots at
    0.5 cycles/row. Slots carry (x_hi, x_lo) of the SAME 128 dims against a
    duplicated W_hi (full x precision in one instruction), plus depth-packed
    x_hi chains against W_lo to reconstruct W. x_lo ships for the first
    _DR_LOK k-chunks only (1.75 B/elem); scales undone on the host."""
    E4 = mybir.dt.float8e4
    DR = mybir.MatmulPerfMode.DoubleRow
    LOK = _DR_LOK
    HIK = KC - LOK
    nc = bacc.Bacc(None, target_bir_lowering=False)

    xA = nc.dram_tensor("xA", [128, T * LOK * 2 * 128], E4, kind="ExternalInput")
    xB = nc.dram_tensor("xB", [128, T * HIK * 128], E4, kind="ExternalInput")
    wA = nc.dram_tensor("wA", [128, LOK * A], E4, kind="ExternalInput")
    wB = nc.dram_tensor("wB", [128, HIK * A], E4, kind="ExternalInput")
    wC = nc.dram_tensor("wC", [128, (KC // 2) * 2 * A], E4, kind="ExternalInput")
    out = nc.dram_tensor("out", [128, T * A], _F16, kind="ExternalOutput")

    with ExitStack() as ctx:
        tc = ctx.enter_context(tile.TileContext(nc))
        const = ctx.enter_context(tc.tile_pool(name="const", bufs=1))
        xin = ctx.enter_context(tc.tile_pool(name="xin", bufs=_BUFS_XIN))
        po = ctx.enter_context(tc.tile_pool(name="po", bufs=_BUFS_PO, space="PSUM"))
        pod = (
            ctx.enter_context(tc.tile_pool(name="pod", bufs=1, space="PSUM"))
            if _MIDWARM > 0
            else None
        )
        ob = ctx.enter_context(tc.tile_pool(name="ob", bufs=_BUFS_OB))

        lgroups = _groups(_LHEAD, _LMID, _LTAIL)
        sgroups = _groups(_SHEAD, _SMID, _STAIL)

        tiles = {}
        btiles = {}

        def load_group(row0, g):
            tA = xin.tile([128, g, LOK, 2, 128], E4, tag=f"xa{g}")
            nc.sync.dma_start(
                tA[:],
                xA[:, bass.ds(row0 * LOK * 256, g * LOK * 256)].rearrange(
                    "p (t k s c) -> p t k s c", t=g, k=LOK, s=2, c=128
                ),
            )
            for t in range(g):
                tiles[row0 + t] = (tA, t)

        def load_b(row0, n):
            if n <= 0:
                return
            # the tiny hi-only k6..7 regions ride in bulk DMAs (held in SBUF
            # for the whole run) instead of wasting a HWDGE slot per group
            tB = const.tile([128, n, HIK, 128], E4, tag=f"xball{row0}")
            nc.sync.dma_start(
                tB[:],
                xB[:, bass.ds(row0 * HIK * 128, n * HIK * 128)].rearrange(
                    "p (t k c) -> p t k c", t=n, k=HIK, c=128
                ),
            )
            for t in range(n):
                btiles[row0 + t] = (tB, t)

        load_group(*lgroups[0])
        load_b(0, _BHEAD)
        wA_sb = const.tile([128, LOK, 1, A], E4)
        nc.sync.dma_start(wA_sb[:], wA.rearrange("p (k s a) -> p k s a", k=LOK, s=1))
        wB_sb = const.tile([128, HIK, A], E4)
        nc.sync.dma_start(wB_sb[:], wB.rearrange("p (k a) -> p k a", k=HIK))
        wC_sb = const.tile([128, KC // 2, 2, A], E4)
        nc.sync.dma_start(wC_sb[:], wC.rearrange("p (j s a) -> p j s a", j=KC // 2, s=2))

        if _WARM > 0:
            zwarm = const.tile([128, A], _F16)
            nc.vector.memset(zwarm[:], 0.0)
            pw = po.tile([128, A], _F32, tag="p_out")
            for _ in range(_WARM):
                nc.tensor.matmul(
                    pw[:], lhsT=zwarm[:, ts(0, 128)], rhs=zwarm[:],
                    start=True, stop=True,
                )

        for gi, (row0, g) in enumerate(lgroups[1:], start=1):
            if gi == _BPOS:
                load_b(_BHEAD, T - _BHEAD)
            load_group(row0, g)
        if len(lgroups) <= _BPOS:
            load_b(_BHEAD, T - _BHEAD)

        for si, (srow0, sg) in enumerate(sgroups):
            og = ob.tile([128, sg, A], _F16, tag=f"ob{sg}")
            for j in range(sg):
                trow = srow0 + j
                tA, t = tiles[trow]
                tB, tb = btiles[trow]
                p_out = po.tile([128, A], _F32, tag="p_out")
                # (x_hi + x_lo) @ W_hi for k0..LOK-1: slots are hi/lo of the
                # same dims, rhs slots duplicate W_hi
                for k in range(LOK):
                    nc.tensor.matmul(
                        p_out[:], lhsT=tA[:, t, k, :, :],
                        rhs=wA_sb[:, k, :, :].broadcast_to([128, 2, A]),
                        start=(k == 0), stop=False, perf_mode=DR,
                    )
                # x_hi @ W_hi for the remaining k, depth-packed 2 chunks/inst
                nc.tensor.matmul(
                    p_out[:], lhsT=tB[:, tb, :, :], rhs=wB_sb[:],
                    start=False, stop=False, perf_mode=DR,
                )
                # x_hi @ W_lo over all k, depth-packed pairs
                for pr in range(KC // 2):
                    if 2 * pr + 1 < LOK:
                        lhsT = tA[:, t, ts(pr, 2), 0, :]
                    else:
                        lhsT = tB[:, tb, :, :]
                    nc.tensor.matmul(
                        p_out[:], lhsT=lhsT, rhs=wC_sb[:, pr, :, :],
                        start=False, stop=(pr == KC // 2 - 1), perf_mode=DR,
                    )
                nc.vector.tensor_copy(out=og[:, j, :], in_=p_out[:])
            dst = out[:, bass.ds(srow0 * A, sg * A)]
            ring = nc.sync if (_LAST_SP and si == len(sgroups) - 1) else nc.scalar
            ring.dma_start(dst.rearrange("p (t a) -> p t a", t=sg), og[:])
            if _MIDWARM > 0 and si < len(sgroups) - 1:
                # dependency-free matmuls run whenever the PE would idle
                # waiting for the next loads, keeping the p-state ramp hot
                pd = pod.tile([128, A], _F32, tag="pd")
                for _ in range(_MIDWARM):
                    nc.tensor.matmul(
                        pd[:], lhsT=zwarm[:, ts(0, 128)], rhs=zwarm[:],
                        start=True, stop=True,
                    )

    nc.finalize()
    return nc


_NC_CACHE = None


def _get_nc():
    global _NC_CACHE
    if _NC_CACHE is None:
        _NC_CACHE = _build_nc()
    return _NC_CACHE


def _fold_weights(geodesic_weights: np.ndarray, W: np.ndarray) -> np.ndarray:
    """W' = W @ blockdiag(L(tanh(g))^T per 4-group), in float64."""
    q = np.tanh(geodesic_weights.astype(np.float64))[0]  # [N, 4]
    w_, i_, j_, k_ = q[:, 0], q[:, 1], q[:, 2], q[:, 3]
    n = q.shape[0]
    M = np.empty((n, 4, 4), dtype=np.float64)  # y_r = sum_s M[n, r, s] x_s
    M[:, 0] = np.stack([w_, -i_, -j_, -k_], axis=-1)
    M[:, 1] = np.stack([i_, w_, -k_, j_], axis=-1)
    M[:, 2] = np.stack([j_, k_, w_, -i_], axis=-1)
    M[:, 3] = np.stack([k_, -j_, i_, w_], axis=-1)
    W4 = W.astype(np.float64).reshape(A, n, 4)  # [a, n, r]
    Wp = np.einsum("anr,nrs->ans", W4, M).reshape(A, D)
    return Wp  # [a, d] float64


def _pack_dr(x, Wp):
    """Host packing for the DoubleRow kernel (see _build_nc_dr)."""
    E4 = ml_dtypes.float8_e4m3
    LOK = _DR_LOK
    HIK = KC - LOK
    xs = x * np.float32(_DR_SX)
    xh = xs.astype(E4).astype(np.float32)
    xl = (xs - xh).astype(E4)
    xh8 = xh.astype(E4)
    # [core, t, c, k, p]
    xh_r = xh8.reshape(N_CORES, T, 128, KC, 128)
    xl_r = xl.reshape(N_CORES, T, 128, KC, 128)
    # A: [core, p, t, k(<LOK), s, c]
    xA = np.ascontiguousarray(
        np.stack([xh_r[:, :, :, :LOK, :], xl_r[:, :, :, :LOK, :]], axis=4)
        .transpose(0, 5, 1, 3, 4, 2)
        .reshape(N_CORES, 128, T * LOK * 2 * 128)
    )
    xB = np.ascontiguousarray(
        xh_r[:, :, :, LOK:, :]
        .transpose(0, 4, 1, 3, 2)
        .reshape(N_CORES, 128, T * HIK * 128)
    )
    ws = (Wp * _DR_SW).astype(np.float32)
    wh = ws.astype(E4).astype(np.float32)
    wl = (ws - wh).astype(E4)
    wh8 = wh.astype(E4)
    whk = wh8.T.reshape(KC, 128, A)  # [k, p, a]
    wlk = wl.T.reshape(KC, 128, A)
    wA = np.ascontiguousarray(
        whk[:LOK].transpose(1, 0, 2).reshape(128, LOK * A)
    )
    wB = np.ascontiguousarray(
        whk[LOK:].transpose(1, 0, 2).reshape(128, HIK * A)
    )
    wC = np.ascontiguousarray(
        wlk.reshape(KC // 2, 2, 128, A).transpose(2, 0, 1, 3).reshape(128, -1)
    )
    return xA, xB, wA, wB, wC


def kernel(x, geodesic_weights, W, b, **_unused):
    x = np.asarray(x, dtype=np.float32)
    Wp = _fold_weights(np.asarray(geodesic_weights), np.asarray(W))
    if _XDT == "dr":
        xA, xB, wA, wB, wC = _pack_dr(x, Wp.astype(np.float32))
        nc = _get_nc()
        in_maps = [
            {"xA": xA[c], "xB": xB[c], "wA": wA, "wB": wB, "wC": wC}
            for c in range(N_CORES)
        ]
        res = run_bass_kernel_spmd(
            nc,
            in_maps,
            core_ids=list(range(N_CORES)),
            trace=bool(int(os.environ.get("KERNEL_TRACE", "0"))),
        )
        bf = np.asarray(b, dtype=np.float32)
        inv = np.float32(1.0 / (_DR_SX * _DR_SW))
        out = np.empty((B_FULL, A), dtype=np.float32)
        for c, r in enumerate(res.results):
            o = r["out"].reshape(128, T, A).transpose(1, 0, 2).reshape(B_SHARD, A)
            out[c * B_SHARD : (c + 1) * B_SHARD] = o.astype(np.float32) * inv + bf
        return out
    if _XDT == "f8":
        # global scale keeps x in e3m4's normal range; undone inside W'
        x_dev_full = (x * _X8_SCALE).astype(ml_dtypes.float8_e3m4)
        Wp = Wp / _X8_SCALE
    else:
        x_dev_full = x.astype(np.float16)
    # device layouts (see _build_nc)
    w_dev = np.ascontiguousarray(
        Wp.T.reshape(KC, 128, A).transpose(1, 0, 2).reshape(128, KC * A)
    ).astype(np.float16)
    # x[core] -> [p, t, k, c]
    xt = np.ascontiguousarray(
        x_dev_full.reshape(N_CORES, T, 128, KC, 128)
        .transpose(0, 4, 1, 3, 2)
        .reshape(N_CORES, 128, T * KC * 128)
    )

    nc = _get_nc()
    in_maps = [{"x": xt[c], "w": w_dev} for c in range(N_CORES)]
    if _W8TILES > 0:
        w8_dev = (w_dev.astype(np.float32) * _W8SCALE).astype(
            ml_dtypes.float8_e3m4
        )
        for m in in_maps:
            m["w8"] = w8_dev
    res = run_bass_kernel_spmd(
        nc,
        in_maps,
        core_ids=list(range(N_CORES)),
        trace=bool(int(os.environ.get("KERNEL_TRACE", "0"))),
    )
    bf = np.asarray(b, dtype=np.float32)
    out = np.empty((B_FULL, A), dtype=np.float32)
    for c, r in enumerate(res.results):
        o = r["out"].reshape(128, T, A).transpose(1, 0, 2).reshape(B_SHARD, A)
        out[c * B_SHARD : (c + 1) * B_SHARD] = o.astype(np.float32) + bf
    return out
